# revision 3
# baseline (speedup 1.0000x reference)
"""Additive attention (d2l-style) on 8 Trainium2 NeuronCores — low-rank scores.

reference math per batch b (B=8, Q=256, K=512, D=256, H=128):
    scores[q, k] = sum_h W_v[h] * tanh(qf[h,q] + kf[h,k])
    attn = softmax_k(scores), masked to k < valid_length[b]
    out  = attn @ value

Numerics (unchanged from the 23.2us baseline): a Gaussian-weighted
separable expansion tanh(x+y) ~= sum_t f_t(x) g_t(y) with R=6 terms
(t0,t1 bf16; t2..5 fp8-e4m3 contracted pairwise with DoubleRow matmuls)
turns the score computation into PE matmuls with contraction R*H;
exp on ACT; E^T @ [V | 1] gives numerator|denominator which the host
divides during unshard.  Host prepares the O((Q+K)*H) feature maps.

v2 schedule changes (from perfetto analysis of the baseline):
- The baseline's first real matmul waited on the whole 393KB pk0 DMA
  (sem at ~11.0us).  v2 splits chunk-0's pack by dtype: pk0a (bf16
  U+G0, 197KB) lands first and starts the bf16 score matmuls ~1.7us
  earlier; pk0b (fp8, 197KB) follows for the DoubleRow pair.
- The junk-matmul p-state warmup train now overruns the expected data
  arrival so the PE never idles between junk and real work (a 333ns gap
  in the baseline reset the 0.65->1.2->2.4GHz ramp and pinned the whole
  compute phase at 1.2GHz).
- V ships as [V0] + [V1..3] on the ACT HWDGE queue so EV chunk 0 isn't
  gated on the full 263KB value transfer.
- Each score chunk gets its own PSUM bank (4 sc banks + 2 out + 1 junk
  of 8) so no matmul ever waits WAR on exp reading a recycled bank.
- Output halves cast on different engines (DVE / Pool) and ship on the
  two HWDGE queues in parallel.
"""

import sys
from contextlib import ExitStack

if "/opt/trn_rl_repo" not in sys.path:
    sys.path.insert(0, "/opt/trn_rl_repo")

import numpy as np

B, Q, K, D, H, V = 8, 256, 512, 256, 128, 256
NCORES = 8
R = 6          # separable rank of tanh(x+y)
NCH = K // 128  # key chunks per core (uniform; pads are data, not code)
A_LIM = 8.0
NGRID = 1601
N8 = R - 2     # fp8 components
VW = V + 1

NJUNK = 13     # PE p-state warmup matmuls (tuned against the trace)
JFREE = 128    # junk matmul free dim

_NC_CACHE = None
_BASIS = None
_LAST_RESULTS = None


def _basis():
    """Gaussian-weighted separable expansion tanh(x+y) ~= sum_t f_t(x)g_t(y)."""
    global _BASIS
    if _BASIS is None:
        x = np.linspace(-A_LIM, A_LIM, NGRID)
        w = np.exp(-0.5 * x**2) / np.sqrt(2 * np.pi) + 1e-4
        sw = np.sqrt(w)
        Aw = sw[:, None] * np.tanh(x[:, None] + x[None, :]) * sw[None, :]
        lam, phi = np.linalg.eigh(Aw)  # symmetric kernel
        idx = np.argsort(-np.abs(lam))[:R]
        lam, phi = lam[idx], phi[:, idx]
        ftab = phi * np.sqrt(np.abs(lam))[None, :] / sw[:, None]
        gtab = ftab * np.sign(lam)[None, :]
        _BASIS = (x, ftab, gtab)
    return _BASIS


def _build():
    from concourse import bacc, mybir, tile

    f32 = mybir.dt.float32
    bf16 = mybir.dt.bfloat16
    f8 = mybir.dt.float8e4

    nc = bacc.Bacc(
        "TRN2",
        target_bir_lowering=False,
        debug=False,
        enable_asserts=False,
        num_devices=NCORES,
    )

    # DRAM inputs, split so completion semaphores match compute need order.
    # Sync HWDGE queue: pk0a -> pk0b -> g1 -> g2 -> g3
    # ACT HWDGE queue:  v0 -> v123  (+ half the output at the end)
    pk0a_d = nc.dram_tensor("pk0a", [128, 2 * Q + 2 * 128], bf16, kind="ExternalInput")
    pk0b_d = nc.dram_tensor("pk0b", [128, N8 * Q + N8 * 128], f8, kind="ExternalInput")
    g_d = [
        nc.dram_tensor(f"g{c}", [128, 2 * 128 + N8 * 128 // 2], bf16,
                       kind="ExternalInput")
        for c in range(1, NCH)
    ]
    v0_d = nc.dram_tensor("v0", [128, VW], bf16, kind="ExternalInput")
    v123_d = nc.dram_tensor("v123", [128, (NCH - 1) * VW], bf16,
                            kind="ExternalInput")
    out_d = nc.dram_tensor("out", [128, 2 * VW], bf16, kind="ExternalOutput")

    Exp = mybir.ActivationFunctionType.Exp
    DR = mybir.MatmulPerfMode.DoubleRow

    with tile.TileContext(nc) as tc, ExitStack() as ctx:
        sb = ctx.enter_context(tc.tile_pool(name="sb", bufs=1))
        ps = ctx.enter_context(tc.tile_pool(name="ps", bufs=1, space="PSUM"))

        pk0a_t = sb.tile([128, 2 * Q + 2 * 128], bf16, tag="pk0a", name="pk0a")
        pk0b_t = sb.tile([128, N8 * Q + N8 * 128], f8, tag="pk0b", name="pk0b")
        g_t = [
            sb.tile([128, 2 * 128 + N8 * 128 // 2], bf16, tag=f"g{c}",
                    name=f"g{c}")
            for c in range(1, NCH)
        ]
        v0_t = sb.tile([128, VW], bf16, tag="v0", name="v0")
        v123_t = sb.tile([128, (NCH - 1) * VW], bf16, tag="v123", name="v123")

        # DMA triggers first in program order so the queues fire ASAP
        nc.sync.dma_start(pk0a_t[:, :], pk0a_d[:, :])
        nc.sync.dma_start(pk0b_t[:, :], pk0b_d[:, :])
        for i in range(NCH - 1):
            nc.sync.dma_start(g_t[i][:, :], g_d[i][:, :])
        nc.scalar.dma_start(v0_t[:, :], v0_d[:, :])
        nc.scalar.dma_start(v123_t[:, :], v123_d[:, :])

        # exp table preload off the critical path
        warm = sb.tile([1, 1], f32, tag="warm")
        nc.gpsimd.memset(warm[:, :], 0.0)
        nc.scalar.activation(warm[:, :], warm[:, :], Exp)

        # PE p-state warmup: junk matmuls cover the DMA lead-in so the clock
        # ramp (0.65 -> 1.2 -> 2.4 GHz, advancing only while the PE stays
        # continuously busy) is as far along as possible when real matmuls
        # start; the train deliberately overruns pk0a's expected arrival
        # because an idle gap resets the ramp.
        junk = sb.tile([128, JFREE], bf16, tag="junk")
        nc.gpsimd.memset(junk[:, :], 0.0)
        jp = ps.tile([128, JFREE], f32, tag="jp")
        for i in range(NJUNK):
            nc.tensor.matmul(
                jp[:, :], junk[:, :], junk[:, :], start=(i == 0),
                stop=(i == NJUNK - 1),
            )

        # logical pieces
        def u_bf(t):
            return pk0a_t[:, t * Q : (t + 1) * Q]

        def g0_bf(t):
            return pk0a_t[:, 2 * Q + t * 128 : 2 * Q + (t + 1) * 128]

        def u_8pair(i):
            sl = pk0b_t[:, 2 * i * Q : 2 * (i + 1) * Q]
            return sl.rearrange("p (two f) -> p two f", two=2)

        def g0_8pair(i):
            off = N8 * Q
            sl = pk0b_t[:, off + i * 256 : off + (i + 1) * 256]
            return sl.rearrange("p (two f) -> p two f", two=2)

        def g_bf(c, t):  # c >= 1
            return g_t[c - 1][:, t * 128 : (t + 1) * 128]

        def g_8pair(c, i):  # c >= 1
            view = g_t[c - 1][:, 2 * 128 :].bitcast(f8)
            sl = view[:, i * 256 : (i + 1) * 256]
            return sl.rearrange("p (two f) -> p two f", two=2)

        def v_sl(c):
            if c == 0:
                return v0_t[:, :]
            return v123_t[:, (c - 1) * VW : c * VW]

        o_tiles = [
            ps.tile([128, VW], f32, tag=f"o{h2}", name=f"o{h2}")
            for h2 in range(2)
        ]
        sc_tiles = [
            ps.tile([128, Q], f32, tag=f"sc{c}", name=f"sc{c}")
            for c in range(NCH)
        ]

        def emit_scores(c, split_exp=False):
            sc = sc_tiles[c]
            for t in range(2):
                gb = g0_bf(t) if c == 0 else g_bf(c, t)
                nc.tensor.matmul(
                    sc[:, :], gb, u_bf(t), start=(t == 0), stop=False
                )
            for i in range(N8 // 2):
                g8 = g0_8pair(i) if c == 0 else g_8pair(c, i)
                nc.tensor.matmul(
                    sc[:, :], g8, u_8pair(i), start=False,
                    stop=(i == N8 // 2 - 1), perf_mode=DR,
                )
            et = sb.tile([128, Q], bf16, tag=f"et{c}", name=f"et{c}")
            if split_exp:
                # last chunk: per-half exp so EV/cast/out of h0 fire earlier
                nc.scalar.activation(et[:, :128], sc[:, :128], Exp)
                nc.scalar.activation(et[:, 128:], sc[:, 128:], Exp)
            else:
                nc.scalar.activation(et[:, :], sc[:, :], Exp)
            return et

        def emit_ev(c, et):
            for h2 in range(2):
                nc.tensor.matmul(
                    o_tiles[h2][:, :],
                    et[:, h2 * 128 : (h2 + 1) * 128],
                    v_sl(c),
                    start=(c == 0),
                    stop=(c == NCH - 1),
                )

        # pipeline: emit scores(c+1) before EV(c) so PE never waits on ACT
        pending = None
        for c in range(NCH):
            et = emit_scores(c, split_exp=(c == NCH - 1))
            if pending is not None:
                emit_ev(*pending)
            pending = (c, et)
        emit_ev(*pending)

        # ship raw numerator|denominator; the host divides during unshard.
        # Halves cast on different engines (ACT copy for h0 — it finishes
        # first and ACT is idle after the last exp; DVE for h1) and ship on
        # both HWDGE queues so the two output paths run in parallel.
        Copy = mybir.ActivationFunctionType.Copy
        osb = sb.tile([128, 2 * VW], bf16, tag="osb")
        nc.scalar.activation(osb[:, :VW], o_tiles[0][:, :], Copy)
        nc.scalar.dma_start(out_d[:, :VW], osb[:, :VW])
        nc.vector.tensor_copy(osb[:, VW:], o_tiles[1][:, :])
        nc.sync.dma_start(out_d[:, VW:], osb[:, VW:])

    nc.compile()
    return nc


def _feat(tab, x, pts):
    out = np.empty(pts.shape + (R,), dtype=np.float32)
    for t in range(R):
        out[..., t] = np.interp(pts, x, tab[:, t])
    return out


def _prep_in_maps(queries, key, value, W_k, W_q, W_v, Ls):
    import ml_dtypes

    bf16 = ml_dtypes.bfloat16
    f8 = ml_dtypes.float8_e4m3fn
    x, ftab, gtab = _basis()
    wv = W_v[0].astype(np.float32)

    # host projections (tiny, <1% of FLOPs — same as baseline)
    qf = np.einsum("hd,bqd->bqh", W_q, queries, optimize=True)
    kf = np.einsum("hd,bkd->bkh", W_k, key, optimize=True)

    def as_bf(a8):  # view fp8 bytes as bf16 carrier elements
        return a8.view(np.uint8).reshape(H, -1, 2).view(np.uint16).reshape(
            H, -1
        ).view(bf16)

    in_maps = []
    for b in range(B):
        L = int(Ls[b])
        # U[h, t*Q + q] = wv[h] * f_t(qf[b,q,h])
        fq = _feat(ftab, x, qf[b])                      # [Q, H, R]
        U = (fq * wv[None, :, None]).transpose(1, 2, 0)  # [H, R, Q]
        U = np.ascontiguousarray(U.reshape(H, R * Q))
        U_bf = U[:, : 2 * Q].astype(bf16)
        U_8 = U[:, 2 * Q :].astype(f8)

        # G[c, h, t*128 + j] = g_t(kf[b, c*128+j, h]), zero for k >= L;
        # pad chunks duplicate chunk 0 (finite scores under exp, V there is 0)
        gk = _feat(gtab, x, kf[b])                      # [K, H, R]
        gk[L:] = 0.0
        G = gk.transpose(1, 2, 0).reshape(H, R, NCH, 128)
        G = np.ascontiguousarray(G.transpose(2, 0, 1, 3)).reshape(
            NCH, H, R * 128
        )
        nreal = max(1, -(-L // 128))
        G[nreal:] = G[0]
        G_bf = G[:, :, : 2 * 128].astype(bf16)
        G_8 = G[:, :, 2 * 128 :].astype(f8)

        # V chunks with ones column; rows >= L zeroed
        Vv = np.zeros((K, VW), dtype=np.float32)
        Vv[:L, :V] = value[b, :L]
        Vv[:L, V] = 1.0
        Vv = Vv.reshape(NCH, 128, VW).astype(bf16)

        m = {
            "pk0a": np.concatenate([U_bf, G_bf[0]], axis=1),
            "pk0b": np.concatenate([U_8, G_8[0]], axis=1),
            "v0": np.ascontiguousarray(Vv[0]),
            "v123": np.ascontiguousarray(
                Vv[1:].transpose(1, 0, 2).reshape(128, (NCH - 1) * VW)
            ),
        }
        for c in range(1, NCH):
            m[f"g{c}"] = np.concatenate([G_bf[c], as_bf(G_8[c])], axis=1)
        in_maps.append(m)
    return in_maps


def kernel(queries, key, value, W_k, W_q, W_v, valid_length):
    global _NC_CACHE, _LAST_RESULTS
    queries = np.asarray(queries, dtype=np.float32)
    key = np.asarray(key, dtype=np.float32)
    value = np.asarray(value, dtype=np.float32)
    W_k = np.asarray(W_k, dtype=np.float32)
    W_q = np.asarray(W_q, dtype=np.float32)
    W_v = np.asarray(W_v, dtype=np.float32)
    Ls = tuple(int(x) for x in np.asarray(valid_length).reshape(-1))
    assert len(Ls) == B and all(1 <= L <= K for L in Ls)

    if _NC_CACHE is None:
        _NC_CACHE = _build()
    nc = _NC_CACHE

    in_maps = _prep_in_maps(queries, key, value, W_k, W_q, W_v, Ls)

    from concourse.bass_utils import run_bass_kernel_spmd

    res = run_bass_kernel_spmd(nc, in_maps, core_ids=list(range(NCORES)))
    _LAST_RESULTS = res

    out = np.empty((B, Q, V), dtype=np.float32)
    for b in range(NCORES):
        raw = res.results[b]["out"].astype(np.float32).reshape(128, 2, VW)
        raw = raw.transpose(1, 0, 2).reshape(Q, VW)
        out[b] = raw[:, :V] / raw[:, V : V + 1]
    return out


# revision 4
# speedup vs baseline: 1.0993x; 1.0993x over previous
"""Additive attention (d2l-style) on 8 Trainium2 NeuronCores — low-rank scores.

reference math per batch b (B=8, Q=256, K=512, D=256, H=128):
    scores[q, k] = sum_h W_v[h] * tanh(qf[h,q] + kf[h,k])
    attn = softmax_k(scores), masked to k < valid_length[b]
    out  = attn @ value

Numerics (unchanged from the 23.2us baseline): a Gaussian-weighted
separable expansion tanh(x+y) ~= sum_t f_t(x) g_t(y) with R=6 terms
(t0,t1 bf16; t2..5 fp8-e4m3 contracted pairwise with DoubleRow matmuls)
turns the score computation into PE matmuls with contraction R*H;
exp on ACT; E^T @ [V | 1] gives numerator|denominator which the host
divides during unshard.  Host prepares the O((Q+K)*H) feature maps.

v3 schedule (from perfetto analysis of two prior variants):
- The PE dispatches at 1.2GHz until ~3us of *continuous* busy, then
  2.4GHz; any idle gap resets the ramp.  A junk-matmul train fills the
  DMA lead-in and deliberately overruns the first pack's arrival; junk
  matmuls are also interleaved at points where the real stream would
  otherwise briefly stall (fp8 pack / g1 arrival, last exp), so the PE
  stays busy from ~7.5us to the last EV and runs the bulk of the real
  matmuls at full clock.
- Chunk-0's pack is split by dtype: pk0a (bf16 U+G0, 197KB) completes
  ~1us before the monolithic 393KB pk0 did, starting real matmuls
  earlier.  pk0b (fp8) follows on the same queue; g1..g3 behind it.
- The V stream is delayed behind dummy ACT ops so its packets don't
  steal DMA-engine bandwidth from pk0a/pk0b/g1 (the completion of which
  gates the PE).  V ships as [V0]+[V1..3] so EV chunk 0 only waits on
  66KB.
- Each score chunk gets its own PSUM bank (4 sc + 2 out + 1 junk of 8)
  so no matmul waits WAR on exp reading a recycled bank.
- Output halves cast on different engines (ACT for h0, DVE for h1) and
  ship concurrently on the two HWDGE queues.
"""

import sys
from contextlib import ExitStack

if "/opt/trn_rl_repo" not in sys.path:
    sys.path.insert(0, "/opt/trn_rl_repo")

import numpy as np

B, Q, K, D, H, V = 8, 256, 512, 256, 128, 256
NCORES = 8
R = 6          # separable rank of tanh(x+y)
NCH = K // 128  # key chunks per core (uniform; pads are data, not code)
A_LIM = 8.0
NGRID = 1601
N8 = R - 2     # fp8 components
VW = V + 1

# schedule tuning knobs (tuned against perfetto traces)
NJUNK = 12     # PE warmup matmuls (free dim 256, ~213ns each at 1.2GHz)
NNOP = 4       # dummy ACT ops delaying the V triggers (~450ns each)
JPAD_DR = 2    # junk between chunk-0 bf16 and fp8 score matmuls
JPAD_SC1 = 2   # junk before chunk-1 scores (g1 arrival jitter)
JPAD_EV3 = 1   # junk before the last EV pair (exp3 wait)

_NC_CACHE = None
_BASIS = None
_LAST_RESULTS = None


def _basis():
    """Gaussian-weighted separable expansion tanh(x+y) ~= sum_t f_t(x)g_t(y)."""
    global _BASIS
    if _BASIS is None:
        x = np.linspace(-A_LIM, A_LIM, NGRID)
        w = np.exp(-0.5 * x**2) / np.sqrt(2 * np.pi) + 1e-4
        sw = np.sqrt(w)
        Aw = sw[:, None] * np.tanh(x[:, None] + x[None, :]) * sw[None, :]
        lam, phi = np.linalg.eigh(Aw)  # symmetric kernel
        idx = np.argsort(-np.abs(lam))[:R]
        lam, phi = lam[idx], phi[:, idx]
        ftab = phi * np.sqrt(np.abs(lam))[None, :] / sw[:, None]
        gtab = ftab * np.sign(lam)[None, :]
        _BASIS = (x, ftab, gtab)
    return _BASIS


def _build():
    from concourse import bacc, mybir, tile

    f32 = mybir.dt.float32
    bf16 = mybir.dt.bfloat16
    f8 = mybir.dt.float8e4

    nc = bacc.Bacc(
        "TRN2",
        target_bir_lowering=False,
        debug=False,
        enable_asserts=False,
        num_devices=NCORES,
    )

    # DRAM inputs.  Sync HWDGE queue: pk0a -> pk0b -> g1 -> g2 -> g3
    # (completion order matches compute need order).  ACT HWDGE queue,
    # delayed: v0 -> v123 (+ half the output at the end).
    pk0a_d = nc.dram_tensor("pk0a", [128, 2 * Q + 2 * 128], bf16, kind="ExternalInput")
    pk0b_d = nc.dram_tensor("pk0b", [128, N8 * Q + N8 * 128], f8, kind="ExternalInput")
    g_d = [
        nc.dram_tensor(f"g{c}", [128, 2 * 128 + N8 * 128 // 2], bf16,
                       kind="ExternalInput")
        for c in range(1, NCH)
    ]
    v0_d = nc.dram_tensor("v0", [128, VW], bf16, kind="ExternalInput")
    v123_d = nc.dram_tensor("v123", [128, (NCH - 1) * VW], bf16,
                            kind="ExternalInput")
    out_d = nc.dram_tensor("out", [128, 2 * VW], bf16, kind="ExternalOutput")

    Exp = mybir.ActivationFunctionType.Exp
    Copy = mybir.ActivationFunctionType.Copy
    DR = mybir.MatmulPerfMode.DoubleRow

    with tile.TileContext(nc) as tc, ExitStack() as ctx:
        sb = ctx.enter_context(tc.tile_pool(name="sb", bufs=1))
        ps = ctx.enter_context(tc.tile_pool(name="ps", bufs=1, space="PSUM"))

        pk0a_t = sb.tile([128, 2 * Q + 2 * 128], bf16, tag="pk0a", name="pk0a")
        pk0b_t = sb.tile([128, N8 * Q + N8 * 128], f8, tag="pk0b", name="pk0b")
        g_t = [
            sb.tile([128, 2 * 128 + N8 * 128 // 2], bf16, tag=f"g{c}",
                    name=f"g{c}")
            for c in range(1, NCH)
        ]
        v0_t = sb.tile([128, VW], bf16, tag="v0", name="v0")
        v123_t = sb.tile([128, (NCH - 1) * VW], bf16, tag="v123", name="v123")

        # score-pack DMA triggers first in program order (Sync queue)
        nc.sync.dma_start(pk0a_t[:, :], pk0a_d[:, :])
        nc.sync.dma_start(pk0b_t[:, :], pk0b_d[:, :])
        for i in range(NCH - 1):
            nc.sync.dma_start(g_t[i][:, :], g_d[i][:, :])

        # junk-tile memsets on the otherwise-idle Pool engine
        junk = sb.tile([128, 256], bf16, tag="junk")
        nc.gpsimd.memset(junk[:, :], 0.0)
        warm = sb.tile([1, 1], f32, tag="warm")
        nc.gpsimd.memset(warm[:, :], 0.0)
        nopo = sb.tile([1, 256], f32, tag="nopo")

        # ACT: exp table preload, then dummy ops that *delay* the V triggers
        # so V packets don't contend with pk0a/pk0b/g1 (whose completion
        # gates the PE); V lands right behind them, well before EV needs it.
        nc.scalar.activation(warm[:, :], warm[:, :], Exp)
        for _ in range(NNOP):
            nc.scalar.activation(nopo[:, :], junk[0:1, :], Exp)
        nc.scalar.dma_start(v0_t[:, :], v0_d[:, :])
        nc.scalar.dma_start(v123_t[:, :], v123_d[:, :])

        # PE p-state warmup: junk matmuls cover the DMA lead-in; the clock
        # ramp (1.2 -> 2.4 GHz after ~3us) only advances while the PE stays
        # continuously busy, and an idle gap resets it, so the train overruns
        # pk0a's expected arrival.
        jp = ps.tile([128, 256], f32, tag="jp")
        for i in range(NJUNK):
            nc.tensor.matmul(
                jp[:, :], junk[:, :128], junk[:, :], start=(i == 0),
                stop=(i == NJUNK - 1),
            )

        def jpad(n):
            for _ in range(n):
                nc.tensor.matmul(jp[:, :], junk[:, :128], junk[:, :],
                                 start=True, stop=True)

        # logical pieces
        def u_bf(t):
            return pk0a_t[:, t * Q : (t + 1) * Q]

        def g0_bf(t):
            return pk0a_t[:, 2 * Q + t * 128 : 2 * Q + (t + 1) * 128]

        def u_8pair(i):
            sl = pk0b_t[:, 2 * i * Q : 2 * (i + 1) * Q]
            return sl.rearrange("p (two f) -> p two f", two=2)

        def g0_8pair(i):
            off = N8 * Q
            sl = pk0b_t[:, off + i * 256 : off + (i + 1) * 256]
            return sl.rearrange("p (two f) -> p two f", two=2)

        def g_bf(c, t):  # c >= 1
            return g_t[c - 1][:, t * 128 : (t + 1) * 128]

        def g_8pair(c, i):  # c >= 1
            view = g_t[c - 1][:, 2 * 128 :].bitcast(f8)
            sl = view[:, i * 256 : (i + 1) * 256]
            return sl.rearrange("p (two f) -> p two f", two=2)

        def v_sl(c):
            if c == 0:
                return v0_t[:, :]
            return v123_t[:, (c - 1) * VW : c * VW]

        o_tiles = [
            ps.tile([128, VW], f32, tag=f"o{h2}", name=f"o{h2}")
            for h2 in range(2)
        ]
        sc_tiles = [
            ps.tile([128, Q], f32, tag=f"sc{c}", name=f"sc{c}")
            for c in range(NCH)
        ]

        def emit_scores(c, split_exp=False):
            sc = sc_tiles[c]
            for t in range(2):
                gb = g0_bf(t) if c == 0 else g_bf(c, t)
                nc.tensor.matmul(
                    sc[:, :], gb, u_bf(t), start=(t == 0), stop=False
                )
            if c == 0:
                jpad(JPAD_DR)  # pk0b lands ~0.5us after pk0a
            for i in range(N8 // 2):
                g8 = g0_8pair(i) if c == 0 else g_8pair(c, i)
                nc.tensor.matmul(
                    sc[:, :], g8, u_8pair(i), start=False,
                    stop=(i == N8 // 2 - 1), perf_mode=DR,
                )
            et = sb.tile([128, Q], bf16, tag=f"et{c}", name=f"et{c}")
            if split_exp:
                # last chunk: per-half exp so EV/cast/out of h0 fire earlier
                nc.scalar.activation(et[:, :128], sc[:, :128], Exp)
                nc.scalar.activation(et[:, 128:], sc[:, 128:], Exp)
            else:
                nc.scalar.activation(et[:, :], sc[:, :], Exp)
            return et

        def emit_ev(c, et):
            for h2 in range(2):
                nc.tensor.matmul(
                    o_tiles[h2][:, :],
                    et[:, h2 * 128 : (h2 + 1) * 128],
                    v_sl(c),
                    start=(c == 0),
                    stop=(c == NCH - 1),
                )

        # pipeline: emit scores(c+1) before EV(c) so PE never waits on ACT
        pending = None
        for c in range(NCH):
            if c == 1:
                jpad(JPAD_SC1)  # g1 arrival jitter
            if c == NCH - 1:
                jpad(JPAD_EV3)
            et = emit_scores(c, split_exp=(c == NCH - 1))
            if pending is not None:
                emit_ev(*pending)
            pending = (c, et)
        emit_ev(*pending)

        # ship raw numerator|denominator; the host divides during unshard.
        # Halves cast on different engines (ACT copy for h0 — it finishes
        # first and ACT is idle after the last exp; DVE for h1) and ship on
        # both HWDGE queues so the two output paths run in parallel.
        osb = sb.tile([128, 2 * VW], bf16, tag="osb")
        nc.scalar.activation(osb[:, :VW], o_tiles[0][:, :], Copy)
        nc.scalar.dma_start(out_d[:, :VW], osb[:, :VW])
        nc.vector.tensor_copy(osb[:, VW:], o_tiles[1][:, :])
        nc.sync.dma_start(out_d[:, VW:], osb[:, VW:])

    nc.compile()
    return nc


def _feat(tab, x, pts):
    out = np.empty(pts.shape + (R,), dtype=np.float32)
    for t in range(R):
        out[..., t] = np.interp(pts, x, tab[:, t])
    return out


def _prep_in_maps(queries, key, value, W_k, W_q, W_v, Ls):
    import ml_dtypes

    bf16 = ml_dtypes.bfloat16
    f8 = ml_dtypes.float8_e4m3fn
    x, ftab, gtab = _basis()
    wv = W_v[0].astype(np.float32)

    # host projections (tiny, <1% of FLOPs — same as baseline)
    qf = np.einsum("hd,bqd->bqh", W_q, queries, optimize=True)
    kf = np.einsum("hd,bkd->bkh", W_k, key, optimize=True)

    def as_bf(a8):  # view fp8 bytes as bf16 carrier elements
        return a8.view(np.uint8).reshape(H, -1, 2).view(np.uint16).reshape(
            H, -1
        ).view(bf16)

    in_maps = []
    for b in range(B):
        L = int(Ls[b])
        # U[h, t*Q + q] = wv[h] * f_t(qf[b,q,h])
        fq = _feat(ftab, x, qf[b])                      # [Q, H, R]
        U = (fq * wv[None, :, None]).transpose(1, 2, 0)  # [H, R, Q]
        U = np.ascontiguousarray(U.reshape(H, R * Q))
        U_bf = U[:, : 2 * Q].astype(bf16)
        U_8 = U[:, 2 * Q :].astype(f8)

        # G[c, h, t*128 + j] = g_t(kf[b, c*128+j, h]), zero for k >= L;
        # pad chunks duplicate chunk 0 (finite scores under exp, V there is 0)
        gk = _feat(gtab, x, kf[b])                      # [K, H, R]
        gk[L:] = 0.0
        G = gk.transpose(1, 2, 0).reshape(H, R, NCH, 128)
        G = np.ascontiguousarray(G.transpose(2, 0, 1, 3)).reshape(
            NCH, H, R * 128
        )
        nreal = max(1, -(-L // 128))
        G[nreal:] = G[0]
        G_bf = G[:, :, : 2 * 128].astype(bf16)
        G_8 = G[:, :, 2 * 128 :].astype(f8)

        # V chunks with ones column; rows >= L zeroed
        Vv = np.zeros((K, VW), dtype=np.float32)
        Vv[:L, :V] = value[b, :L]
        Vv[:L, V] = 1.0
        Vv = Vv.reshape(NCH, 128, VW).astype(bf16)

        m = {
            "pk0a": np.concatenate([U_bf, G_bf[0]], axis=1),
            "pk0b": np.concatenate([U_8, G_8[0]], axis=1),
            "v0": np.ascontiguousarray(Vv[0]),
            "v123": np.ascontiguousarray(
                Vv[1:].transpose(1, 0, 2).reshape(128, (NCH - 1) * VW)
            ),
        }
        for c in range(1, NCH):
            m[f"g{c}"] = np.concatenate([G_bf[c], as_bf(G_8[c])], axis=1)
        in_maps.append(m)
    return in_maps


def kernel(queries, key, value, W_k, W_q, W_v, valid_length):
    global _NC_CACHE, _LAST_RESULTS
    queries = np.asarray(queries, dtype=np.float32)
    key = np.asarray(key, dtype=np.float32)
    value = np.asarray(value, dtype=np.float32)
    W_k = np.asarray(W_k, dtype=np.float32)
    W_q = np.asarray(W_q, dtype=np.float32)
    W_v = np.asarray(W_v, dtype=np.float32)
    Ls = tuple(int(x) for x in np.asarray(valid_length).reshape(-1))
    assert len(Ls) == B and all(1 <= L <= K for L in Ls)

    if _NC_CACHE is None:
        _NC_CACHE = _build()
    nc = _NC_CACHE

    in_maps = _prep_in_maps(queries, key, value, W_k, W_q, W_v, Ls)

    from concourse.bass_utils import run_bass_kernel_spmd

    res = run_bass_kernel_spmd(nc, in_maps, core_ids=list(range(NCORES)))
    _LAST_RESULTS = res

    out = np.empty((B, Q, V), dtype=np.float32)
    for b in range(NCORES):
        raw = res.results[b]["out"].astype(np.float32).reshape(128, 2, VW)
        raw = raw.transpose(1, 0, 2).reshape(Q, VW)
        out[b] = raw[:, :V] / raw[:, V : V + 1]
    return out
